# revision 1
# baseline (speedup 1.0000x reference)
"""Trainium2 Bass kernel for nn_AttentionBlock (GroupNorm + 8-head self-attention
+ projection + residual) on x: [16, 512, 32, 32].

Sharding: data-parallel over batch across 8 NeuronCores (2 batch items/core),
no collectives.

v2 redesign (262us -> 245us, rel err 3.7e-3):
  - Value matmul in fp8e4 DoubleRow perf mode (2 contraction k-tiles per
    pass, 0.5 cycles/row): exp() writes scores directly in fp8 (exp(S-4);
    the -4 shift cancels in softmax and keeps e^S inside fp8e4 range).
    v^T stored fp8 as [ts, 2, 128] per head: col 0 = ones (so the softmax
    denominator accumulates as PSUM row 0, quadrant-aligned for the
    reciprocal), cols 1..63 zero pad (dual-fp8 ldweights needs 16B-aligned
    col blocks; partition ranges must start at 0/32/64/96), cols 64..127 =
    v. Value PE cost per head drops 8192 -> 2048 cycles.
  - GroupNorm rstd via reciprocal seed + 3 Newton steps on DVE (keeps the
    ACT engine's Exp table resident; avoids 4x ACT_TABLE_LOAD).
  - Software-pipelined emission: per head, S-pair matmuls emitted
    back-to-back, value DR pairs delayed two pairs (deps pre-satisfied),
    the last two pairs + evac carried into the NEXT head's stream; qkv of
    the next batch / proj of the previous batch stolen into attention
    m-steps on a fixed pop plan sized so every stolen group is emitted
    before its consumers. proj(1) runs post-loop: stealing it into batch-1
    attention races its PSUM evac (observed flaky corruption).
  - Known limits: PE is the pacer (~192us busy: 123us of matmul columns +
    ldweights/latency overhead); exp on ACT is 143us busy. PSUM is the
    binding resource (8 banks: S 2x2 + value-acc 2 + qkv/proj accs 2).
"""
import math
import sys

sys.path.insert(0, "/opt/trn_rl_repo")

import numpy as np

import concourse.bass as bass  # noqa: F401  (registers types)
import concourse.tile as tile
from concourse import bacc, mybir
from concourse.bass_utils import run_bass_kernel_spmd

AF = mybir.ActivationFunctionType
ALU = mybir.AluOpType
F32 = mybir.dt.float32
I32 = mybir.dt.int32
BF16 = mybir.dt.bfloat16
FP8 = mybir.dt.float8e4

B, C, HH, WW = 16, 512, 32, 32
T = HH * WW            # 1024
NH, CH = 8, 64         # heads, head dim
MV = 128               # padded per-head v^T block: ones | 63 zero | 64 v
VOFF = 64              # v cols start at 64 (quadrant-aligned PSUM rows)
G, CPG = 32, 16        # groupnorm groups, channels per group
EPS = 1e-5
NCORES = 8
BPC = B // NCORES      # 2 batch items per core
P = 128
NCC = C // P           # 4 channel chunks
NTC = T // P           # 8 t chunks (m steps)
NPAIR = NTC // 2       # 4 double-row pairs
NN = T // 512          # 2 n-chunks of 512
DBG = False
EXPSHIFT = -4.0        # exp(S + EXPSHIFT); cancels in softmax, keeps fp8 range
ACCB = 2               # bufs for qkv/proj psum accumulators
EVERY = 2              # steal one work group per EVERY m-steps


def _body(ctx, tc, d):
    nc = tc.nc
    sync = nc.sync

    consts = ctx.enter_context(tc.tile_pool(name="consts", bufs=1))
    xp = ctx.enter_context(tc.tile_pool(name="xp", bufs=2))
    xnp = ctx.enter_context(tc.tile_pool(name="xnp", bufs=2))
    qkp = ctx.enter_context(tc.tile_pool(name="qkp", bufs=1))
    vtp = ctx.enter_context(tc.tile_pool(name="vtp", bufs=2))
    apl = ctx.enter_context(tc.tile_pool(name="apl", bufs=1))
    ep = ctx.enter_context(tc.tile_pool(name="ep", bufs=3))
    smp = ctx.enter_context(tc.tile_pool(name="smp", bufs=2))
    rp = ctx.enter_context(tc.tile_pool(name="rp", bufs=2))
    opl = ctx.enter_context(tc.tile_pool(name="opl", bufs=4))
    ps = ctx.enter_context(tc.tile_pool(name="ps", bufs=2, space="PSUM"))

    # ---- x loads for batch 0 first (startup latency), then constants ----
    xb = {}
    for c in range(NCC):
        xt = xp.tile([P, T], F32, tag=f"x{c}", name=f"x_0_{c}")
        sync.dma_start(xt[:], d["x"][0, c * P:(c + 1) * P, :])
        xb[(0, c)] = xt

    # small GN constants first: gsum/affine need them early, and they must
    # not queue behind the 2MB of weight DMAs
    aux = consts.tile([P, 20], F32)              # bqk[0:8] bproj[8:12] gns[12:16] gnb[16:20]
    sync.dma_start(aux[:], d["aux"][:])
    gmats = consts.tile([P, NCC, G], F32)
    sync.dma_start(gmats[:], d["gmats"][:])
    ematT = consts.tile([G, NCC, P], F32)
    sync.dma_start(ematT[:], d["ematT"][:])
    wqkvT = consts.tile([P, NCC, 3 * C], BF16)   # [128, 4, 1536]
    sync.dma_start(wqkvT[:], d["wqkvT"].rearrange("(cc p) o -> p cc o", p=P))
    wprojT = consts.tile([P, NCC, C], BF16)      # [128, 4, 512]
    sync.dma_start(wprojT[:], d["wprojT"].rearrange("(cc p) o -> p cc o", p=P))
    ebias = consts.tile([P, 1], F32)
    nc.gpsimd.memset(ebias[:], EXPSHIFT)

    xnb, qt, kt, vt, at = {}, {}, {}, {}, {}

    # one-time pad init of all vt physical buffers (tag rotation: 2 bufs/tag):
    # col 0 = ones (softmax denominator weights), cols 1:VOFF = zeros.
    for rep in range(2):
        for mp in range(NPAIR):
            vi = vtp.tile([P, NH, 2, MV], FP8, tag=f"vt{mp}",
                          name=f"vtinit_{rep}_{mp}")
            nc.gpsimd.memset(vi[:, :, :, 0:1], 1.0)
            nc.gpsimd.memset(vi[:, :, :, 1:VOFF], 0.0)

    def load_x(bi):
        for c in range(NCC):
            xt = xp.tile([P, T], F32, tag=f"x{c}", name=f"x_{bi}_{c}")
            sync.dma_start(xt[:], d["x"][bi, c * P:(c + 1) * P, :])
            xb[(bi, c)] = xt

    s12b = {}

    def gn_stats_chunk(bi, c):
        if bi not in s12b:
            s12b[bi] = smp.tile([P, NCC, 2], F32, tag="s12", name=f"s12_{bi}")
        s12 = s12b[bi]
        xnt = xnp.tile([P, T], BF16, tag=f"xn{c}", name=f"xn_{bi}_{c}")
        xnb[(bi, c)] = xnt
        nc.vector.reduce_sum(s12[:, c, 0:1], xb[(bi, c)][:],
                             axis=mybir.AxisListType.X)
        sq = smp.tile([P, T], F32, tag="sq", name=f"sq_{bi}_{c}")
        nc.vector.scalar_tensor_tensor(
            sq[:], xb[(bi, c)][:], 1.0, xb[(bi, c)][:],
            op0=ALU.mult, op1=ALU.mult, accum_out=s12[:, c, 1:2])

    def gn_finish(bi):
        s12 = s12b[bi]
        gsum = ps.tile([G, 2], F32, tag="acc", bufs=ACCB, name=f"gsum_{bi}")
        for c in range(NCC):
            nc.tensor.matmul(gsum[:], gmats[:, c, :], s12[:, c, :],
                             start=(c == 0), stop=(c == NCC - 1))

        ms = smp.tile([G, 4], F32, tag="ms", name=f"ms_{bi}")  # mu, msq, var+eps, mu^2
        nc.vector.tensor_scalar_mul(ms[:, 0:2], gsum[:], 1.0 / (CPG * T))
        nc.vector.tensor_mul(ms[:, 3:4], ms[:, 0:1], ms[:, 0:1])
        nc.vector.scalar_tensor_tensor(ms[:, 2:3], ms[:, 1:2], EPS, ms[:, 3:4],
                                       op0=ALU.add, op1=ALU.subtract)
        # rstd = rsqrt(var+eps) on DVE: reciprocal seed + 3 Newton steps
        # (keeps ACT's Exp table resident -- no table reloads; converges to
        # <1e-5 rel for var in [0.3, 3]; GN var here is ~1).
        musd = smp.tile([G, 2], F32, tag="musd", name=f"musd_{bi}")  # mu, rstd
        nc.vector.tensor_copy(musd[:, 0:1], ms[:, 0:1])
        rs = smp.tile([G, 4], F32, tag="rs", name=f"rs_{bi}")
        # rs: 0=y, 1=hv(0.5v), 2=tmp, 3=w
        nc.vector.reciprocal_approx_fast(rs[:, 0:1], ms[:, 2:3])
        nc.vector.tensor_scalar_mul(rs[:, 1:2], ms[:, 2:3], 0.5)
        for _ in range(3):
            nc.vector.tensor_mul(rs[:, 2:3], rs[:, 0:1], rs[:, 0:1])
            nc.vector.tensor_mul(rs[:, 2:3], rs[:, 2:3], rs[:, 1:2])
            nc.vector.tensor_scalar(out=rs[:, 3:4], in0=rs[:, 2:3],
                                    scalar1=-1.0, scalar2=1.5,
                                    op0=ALU.mult, op1=ALU.add)
            nc.vector.tensor_mul(rs[:, 0:1], rs[:, 0:1], rs[:, 3:4])
        nc.vector.tensor_copy(musd[:, 1:2], rs[:, 0:1])

        for c in range(NCC):
            chan = ps.tile([P, 2], F32, tag="acc", bufs=ACCB, name=f"chan_{bi}_{c}")
            nc.tensor.matmul(chan[:], ematT[:, c, :], musd[:], start=True, stop=True)
            ac = smp.tile([P, 3], F32, tag=f"aff{c}", name=f"aff_{bi}_{c}")  # a, -a, b
            nc.vector.tensor_mul(ac[:, 0:1], aux[:, 12 + c:13 + c], chan[:, 1:2])
            nc.vector.tensor_scalar_mul(ac[:, 1:2], ac[:, 0:1], -1.0)
            nc.vector.scalar_tensor_tensor(
                ac[:, 2:3], chan[:, 0:1], ac[:, 1:2], aux[:, 16 + c:17 + c],
                op0=ALU.mult, op1=ALU.add)
            nc.vector.tensor_scalar(
                out=xnb[(bi, c)][:], in0=xb[(bi, c)][:],
                scalar1=ac[:, 0:1], scalar2=ac[:, 2:3],
                op0=ALU.mult, op1=ALU.add)

    def gn(bi):
        for c in range(NCC):
            gn_stats_chunk(bi, c)
        gn_finish(bi)

    def qkv_groups(bi):
        """Emit-closures, one per psum accumulation group.
        Order: [v0..v7, qk oc0 (4), qk oc1 (4), qk oc2 (4), qk oc3 (4)]."""
        for oc in range(NCC):
            qt[(bi, oc)] = qkp.tile([P, T], BF16, tag=f"q{oc}", bufs=2,
                                    name=f"q_{bi}_{oc}")
            kt[(bi, oc)] = qkp.tile([P, T], BF16, tag=f"k{oc}", bufs=2,
                                    name=f"k_{bi}_{oc}")
        for mp in range(NPAIR):
            vt[(bi, mp)] = vtp.tile([P, NH, 2, MV], FP8, tag=f"vt{mp}",
                                    name=f"vt_{bi}_{mp}")

        def qk_group(dst, base, boff, oc, n):
            def emit():
                acc = ps.tile([P, 512], F32, tag="acc", bufs=ACCB,
                              name=f"qk_{bi}_{base}_{oc}_{n}")
                for kc in range(NCC):
                    nc.tensor.matmul(
                        acc[:],
                        wqkvT[:, kc, base + oc * P:base + (oc + 1) * P],
                        xnb[(bi, kc)][:, n * 512:(n + 1) * 512],
                        start=(kc == 0), stop=(kc == NCC - 1))
                nc.vector.tensor_scalar_add(
                    dst[(bi, oc)][:, n * 512:(n + 1) * 512], acc[:],
                    aux[:, boff + oc:boff + oc + 1])
            return emit

        def v_group(m):
            def emit():
                vtt = vt[(bi, m // 2)]
                j = m % 2
                acc = ps.tile([P, 512], F32, tag="acc", bufs=ACCB,
                              name=f"v_{bi}_{m}")
                for kc in range(NCC):
                    nc.tensor.matmul(acc[:], xnb[(bi, kc)][:, m * P:(m + 1) * P],
                                     wqkvT[:, kc, 2 * C:3 * C],
                                     start=(kc == 0), stop=(kc == NCC - 1))
                nc.vector.tensor_copy(
                    vtt[:, :, j, VOFF:VOFF + CH],
                    acc[:].rearrange("p (h c) -> p h c", c=CH))
            return emit

        groups = [v_group(m) for m in range(NTC)]
        for oc in range(NCC):
            for n in range(NN):
                groups.append(qk_group(qt, 0, 0, oc, n))
                groups.append(qk_group(kt, C, 4, oc, n))
        return groups

    def proj_groups(bi):
        groups = []

        def p_group(oc, n):
            def emit():
                acc = ps.tile([P, 512], F32, tag="acc", bufs=ACCB,
                              name=f"p_{bi}_{oc}_{n}")
                for kc in range(NCC):
                    nc.tensor.matmul(acc[:],
                                     wprojT[:, kc, oc * P:(oc + 1) * P],
                                     at[(bi, kc)][:, n * 512:(n + 1) * 512],
                                     start=(kc == 0), stop=(kc == NCC - 1))
                ot = opl.tile([P, 512], F32, tag="o", name=f"o_{bi}_{oc}_{n}")
                nc.vector.scalar_tensor_tensor(
                    ot[:], acc[:], aux[:, 8 + oc:9 + oc],
                    xb[(bi, oc)][:, n * 512:(n + 1) * 512],
                    op0=ALU.add, op1=ALU.add)
                sync.dma_start(d["out"][bi, oc * P:(oc + 1) * P,
                                        n * 512:(n + 1) * 512], ot[:])
            return emit

        for oc in range(NCC):
            for n in range(NN):
                groups.append(p_group(oc, n))
        return groups

    def qk_upfront_pair(bi, n):
        # q and k oc0 interleaved: consecutive matmuls alternate acc banks
        # (same-bank back-to-back accumulation serializes ~630ns vs ~386ns).
        # Safe here: DVE is idle at startup, so the acc WAR window is empty.
        qa = ps.tile([P, 512], F32, tag="acc", bufs=ACCB, name=f"uq_{bi}_{n}")
        ka = ps.tile([P, 512], F32, tag="acc", bufs=ACCB, name=f"uk_{bi}_{n}")
        for kc in range(NCC):
            nc.tensor.matmul(qa[:], wqkvT[:, kc, 0 * P:(0 + 1) * P],
                             xnb[(bi, kc)][:, n * 512:(n + 1) * 512],
                             start=(kc == 0), stop=(kc == NCC - 1))
            nc.tensor.matmul(ka[:], wqkvT[:, kc, C + 0 * P:C + (0 + 1) * P],
                             xnb[(bi, kc)][:, n * 512:(n + 1) * 512],
                             start=(kc == 0), stop=(kc == NCC - 1))
        nc.vector.tensor_scalar_add(
            qt[(bi, 0)][:, n * 512:(n + 1) * 512], qa[:], aux[:, 0:1])
        nc.vector.tensor_scalar_add(
            kt[(bi, 0)][:, n * 512:(n + 1) * 512], ka[:], aux[:, 4:5])

    def proj_pair(bi, oc):
        # both n-halves of one proj oc interleaved (tail-only; DVE idle there)
        accs = [ps.tile([P, 512], F32, tag="acc", bufs=ACCB,
                        name=f"pp_{bi}_{oc}_{n}") for n in range(NN)]
        for kc in range(NCC):
            for n in range(NN):
                nc.tensor.matmul(accs[n][:],
                                 wprojT[:, kc, oc * P:(oc + 1) * P],
                                 at[(bi, kc)][:, n * 512:(n + 1) * 512],
                                 start=(kc == 0), stop=(kc == NCC - 1))
        for n in range(NN):
            ot = opl.tile([P, 512], F32, tag="o", name=f"op_{bi}_{oc}_{n}")
            nc.vector.scalar_tensor_tensor(
                ot[:], accs[n][:], aux[:, 8 + oc:9 + oc],
                xb[(bi, oc)][:, n * 512:(n + 1) * 512],
                op0=ALU.add, op1=ALU.add)
            sync.dma_start(d["out"][bi, oc * P:(oc + 1) * P,
                                    n * 512:(n + 1) * 512], ot[:])

    def alloc_a(bi):
        for cc in range(NCC):
            at[(bi, cc)] = apl.tile([P, T], BF16, tag=f"a{cc}", bufs=2,
                                    name=f"a_{bi}_{cc}")

    def attn_head(bi, h, work, carry, plan=(1, 1, 1, 1)):
        """One head: S^T + exp stream; value DR pairs delayed one m-step so
        the final pair + evac land in the NEXT head's stream (via `carry`) --
        keeps the PE from stalling on the head's last exp. `plan[mp]` work
        groups are stolen at each odd m."""
        po = (h % 2) * CH
        qh = qt[(bi, h // 2)][po:po + CH, :]
        kh = kt[(bi, h // 2)][po:po + CH, :]
        etp = {}
        st = {}

        def vpair(p):
            if "a" not in st:
                st["a"] = ps.tile([MV, T], F32, tag="aacc", bufs=1,
                                  name=f"aacc_{bi}_{h}")
            for n in range(NN):
                nc.tensor.matmul(
                    st["a"][:, n * 512:(n + 1) * 512],
                    vt[(bi, p)][:, h, :, :],
                    etp[p][:, :, n * 512:(n + 1) * 512],
                    start=(p == 0), stop=(p == NPAIR - 1),
                    perf_mode=mybir.MatmulPerfMode.DoubleRow)

        def evac():
            # row 0 = softmax denominator, rows VOFF:VOFF+64 = unnormalized a
            vpair(NPAIR - 2)
            vpair(NPAIR - 1)
            a96 = rp.tile([MV, T], F32, tag="a96", name=f"a96_{bi}_{h}")
            nc.vector.tensor_copy(a96[:], st["a"][:])
            rr1 = rp.tile([1, T], F32, tag="rr1", name=f"rr_{bi}_{h}")
            nc.vector.reciprocal_approx_fast(rr1[:], a96[0:1, :])
            rb = rp.tile([P, T], F32, tag="rb", name=f"rb_{bi}_{h}")
            nc.gpsimd.partition_broadcast(rb[:], rr1[:])
            nc.vector.tensor_mul(at[(bi, h // 2)][po:po + CH, :],
                                 a96[VOFF:VOFF + CH, :], rb[VOFF:VOFF + CH, :])
            if DBG and bi == 1:
                sync.dma_start(d["dbg_a"][h * MV:(h + 1) * MV, :], a96[:])

        for mp in range(NPAIR):
            etp[mp] = ep.tile([P, 2, T], FP8, tag="et", bufs=6,
                              name=f"e_{bi}_{h}_{mp}")
            sp2 = []
            for j in range(2):
                m = 2 * mp + j
                sps = ps.tile([P, T], F32, tag="s", name=f"s_{bi}_{h}_{m}")
                sp2.append(sps)
                for n in range(NN):
                    nc.tensor.matmul(sps[:, n * 512:(n + 1) * 512],
                                     kh[:, m * P:(m + 1) * P],
                                     qh[:, n * 512:(n + 1) * 512],
                                     start=True, stop=True)
            for j in range(2):
                nc.scalar.activation(etp[mp][:, j, :], sp2[j][:], AF.Exp,
                                     bias=ebias[:])
            for _ in range(plan[mp]):
                if work:
                    work.pop(0)()
            if mp == 0:
                while carry:
                    carry.pop(0)()
            elif mp >= 2:
                vpair(mp - 2)
        carry.append(evac)

    # ---------- software-pipelined emission ----------
    gn(0)
    g0 = qkv_groups(0)
    for n in range(NN):                   # upfront: q/k oc0 (heads 0,1)
        qk_upfront_pair(0, n)
    load_x(1)
    alloc_a(0)
    g1 = qkv_groups(1)
    # steal order: v0..7(b0), qk oc1..3(b0), then all of batch 1
    pending = g0[0:8] + g0[12:24] + g1   # g0[8:12] replaced by upfront pairs
    carry = []
    # pop plans sized so every stolen group lands BEFORE its consumers'
    # emission: qk oc_i fully popped by end of head 2i-1, batch-1 v groups
    # only after gn_finish(1) (emitted after head 2).
    plans0 = {0: (2, 2, 1, 1), 1: (2, 2, 2, 1)}
    for h in range(NH):
        attn_head(0, h, pending, carry, plan=plans0.get(h, (1, 1, 1, 1)))
        if h < 2:
            gn_stats_chunk(1, 2 * h)
            gn_stats_chunk(1, 2 * h + 1)
        if h == 2:
            gn_finish(1)
    alloc_a(1)
    p0 = proj_groups(0)
    pending += p0
    for h in range(NH):
        attn_head(1, h, pending, carry)
    while carry:
        carry.pop(0)()
    for g in pending:
        g()
    for oc in range(NCC):
        proj_pair(1, oc)

def build():
    from contextlib import ExitStack

    nc = bacc.Bacc("TRN2", target_bir_lowering=False, debug=False,
                   num_devices=NCORES)
    d = {
        "x": nc.dram_tensor("x", [BPC, C, T], F32, kind="ExternalInput").ap(),
        "wqkvT": nc.dram_tensor("wqkvT", [C, 3 * C], BF16, kind="ExternalInput").ap(),
        "wprojT": nc.dram_tensor("wprojT", [C, C], BF16, kind="ExternalInput").ap(),
        "aux": nc.dram_tensor("aux", [P, 20], F32, kind="ExternalInput").ap(),
        "gmats": nc.dram_tensor("gmats", [P, NCC, G], F32, kind="ExternalInput").ap(),
        "ematT": nc.dram_tensor("ematT", [G, NCC, P], F32, kind="ExternalInput").ap(),
        "out": nc.dram_tensor("out", [BPC, C, T], F32, kind="ExternalOutput").ap(),
    }
    if DBG:
        d["dbg_a"] = nc.dram_tensor("dbg_a", [NH * MV, T], F32,
                                    kind="ExternalOutput").ap()
        d["dbg_at"] = nc.dram_tensor("dbg_at", [C, T], BF16,
                                     kind="ExternalOutput").ap()
    with tile.TileContext(nc) as tc:
        with ExitStack() as ctx:
            _body(ctx, tc, d)
    nc.compile()
    return nc


_CACHE = {}


def prep_inputs(x, gn_scale, gn_bias, w_qkv, b_qkv, w_proj, b_proj):
    x = np.ascontiguousarray(np.asarray(x, np.float32).reshape(B, C, T))
    gn_scale = np.asarray(gn_scale, np.float32)
    gn_bias = np.asarray(gn_bias, np.float32)
    w_qkv = np.asarray(w_qkv, np.float32)
    b_qkv = np.asarray(b_qkv, np.float32)
    w_proj = np.asarray(w_proj, np.float32)
    b_proj = np.asarray(b_proj, np.float32)

    s = 1.0 / math.sqrt(math.sqrt(CH))
    wqkvT = w_qkv.T.copy()                      # [512, 1536]
    wqkvT[:, :2 * C] *= s                       # fold attention scale into q,k
    wprojT = w_proj.T.copy()                    # [512, 512]

    bqk = (b_qkv[:2 * C] * s).reshape(2 * NCC, P).T          # [128, 8]
    bproj_eff = (b_proj + w_proj @ b_qkv[2 * C:]).reshape(NCC, P).T  # [128, 4]
    gns = gn_scale.reshape(NCC, P).T
    gnb = gn_bias.reshape(NCC, P).T
    aux = np.ascontiguousarray(
        np.concatenate([bqk, bproj_eff, gns, gnb], axis=1), np.float32)

    p = np.arange(P)
    gmats = np.zeros((P, NCC, G), np.float32)
    ematT = np.zeros((G, NCC, P), np.float32)
    for c in range(NCC):
        gmats[p, c, 8 * c + p // CPG] = 1.0
        ematT[8 * c + p // CPG, c, p] = 1.0

    import ml_dtypes
    shared = {"wqkvT": np.ascontiguousarray(wqkvT).astype(ml_dtypes.bfloat16),
              "wprojT": np.ascontiguousarray(wprojT).astype(ml_dtypes.bfloat16),
              "aux": aux, "gmats": gmats, "ematT": ematT}
    in_maps = []
    for ci in range(NCORES):
        m = dict(shared)
        m["x"] = np.ascontiguousarray(x[BPC * ci:BPC * (ci + 1)])
        in_maps.append(m)
    return in_maps


def run(inputs, trace=False, tmpdir=None):
    if "nc" not in _CACHE:
        _CACHE["nc"] = build()
    nc = _CACHE["nc"]
    in_maps = prep_inputs(**inputs)
    kwargs = {}
    if trace:
        kwargs["trace"] = True
    if tmpdir:
        kwargs["tmpdir"] = tmpdir
    res = run_bass_kernel_spmd(nc, in_maps, core_ids=list(range(NCORES)), **kwargs)
    out = np.concatenate([r["out"] for r in res.results], axis=0)
    return out.reshape(B, C, HH, WW), res


def kernel(**inputs):
    return run(inputs)[0]

